# revision 6
# baseline (speedup 1.0000x reference)
"""ClusterNorm1d TRN2 kernel.

Math (per cluster k): mu = mean_b x[b,:,k]; cov = centered second moment;
L = chol(cov + eps I); Z = L^-1 (x - mu).  Output Z transposed back.

Strategy per core (32 clusters): K-sharded across 8 cores, no collectives.
  - stats: bf16 matmuls over a host-prepared [b, (d|1)] tensor, accumulating
    U^T U = [[S, s], [s^T, B]] in fp32 PSUM (32 accumulating matmuls).
  - cov -> W = L^-1 via 4 Newton iterations on the Cholesky manifold:
      P = W A W^T;  C^T = CM o (I - P);  W <- W + C^T^T W
    (CM = triu(1) + 0.5 I).  Converges quadratically; exact-fp32 validated.
  - solve: Z = W x - (W mu) 1^T as float32r matmuls (1 cyc/row @ N=512),
    mean applied as per-partition bias during the PSUM->SBUF copy.
Host supplies x pre-transposed per core as [32, 64, 4096] (f32r) and the
bf16 stats operand [32, 4096, 66] (col 64 = ones, col 65 pad).
"""
import sys
sys.path.insert(0, "/opt/trn_rl_repo")

import numpy as np
import ml_dtypes

import concourse.bass as bass
from concourse import bacc
import concourse.mybir as mybir
import concourse.tile as tile
from concourse.bass_utils import run_bass_kernel_spmd

B, D, K, NCORES = 4096, 64, 256, 8
KL = K // NCORES          # clusters per core
EPS = 1e-4
NB = B // 512             # solve chunks per cluster
AF = mybir.ActivationFunctionType

_cache = {}


def _build_nc():
    nc = bacc.Bacc("TRN2", target_bir_lowering=False, debug=False,
                   num_devices=NCORES)
    d_xs = nc.dram_tensor("xs", [KL, D, B], mybir.dt.float32r,
                          kind="ExternalInput")
    d_xb = nc.dram_tensor("xb", [KL, B, 66], mybir.dt.bfloat16,
                          kind="ExternalInput")
    d_cs = nc.dram_tensor("cs", [D, 4 * D], mybir.dt.float32,
                          kind="ExternalInput")
    d_out = nc.dram_tensor("out", [KL, D, B], mybir.dt.float32,
                           kind="ExternalOutput")

    inv_b = 1.0 / B
    a_cov = 1.0 / (B - 1)
    b_cov = 1.0 / (B * (B - 1.0))

    with tile.TileContext(nc) as tc:
        with tc.tile_pool(name="consts", bufs=1) as consts, \
             tc.tile_pool(name="slab", bufs=2) as slabp, \
             tc.tile_pool(name="upool", bufs=2) as upool, \
             tc.tile_pool(name="zpool", bufs=2) as zpool, \
             tc.tile_pool(name="small", bufs=4) as small, \
             tc.tile_pool(name="wpool", bufs=8) as wpool, \
             tc.tile_pool(name="ps_stat", bufs=2, space="PSUM") as ps_stat, \
             tc.tile_pool(name="ps_small", bufs=4, space="PSUM") as ps_small, \
             tc.tile_pool(name="ps_z", bufs=2, space="PSUM") as ps_z:

            tcs = consts.tile([D, 4 * D], mybir.dt.float32)
            nc.sync.dma_start(out=tcs, in_=d_cs.ap())
            ident = tcs[:, 0:D]
            cmask = tcs[:, D:2 * D]        # triu(1,k=1) + 0.5 I
            chalf = tcs[:, 2 * D:3 * D]    # 0.5 I
            epsi = tcs[:, 3 * D:4 * D]     # EPS * I

            for p in range(KL // 2):
                k0, k1 = 2 * p, 2 * p + 1
                # ---- x slab for the pair: [64, 2*4096] f32r ----
                slab = slabp.tile([D, 2, B], mybir.dt.float32r)
                nc.sync.dma_start(
                    out=slab,
                    in_=d_xs.ap()[k0:k0 + 2].rearrange("c d b -> d c b"))

                zpair = zpool.tile([2 * D, B], mybir.dt.float32)
                outdma_deps = []

                for half, kk in enumerate((k0, k1)):
                    # ---- stats ----
                    ub = upool.tile([128, (B // 128) * 66], mybir.dt.bfloat16)
                    nc.sync.dma_start(
                        out=ub,
                        in_=d_xb.ap()[kk].rearrange("(p j) c -> p (j c)",
                                                    p=128))
                    ps = ps_stat.tile([D + 1, D + 1], mybir.dt.float32)
                    for j in range(B // 128):
                        sl = ub[:, 66 * j:66 * j + 65]
                        nc.tensor.matmul(ps, sl, sl, start=(j == 0),
                                         stop=(j == B // 128 - 1))
                    st = small.tile([D + 1, D + 1], mybir.dt.float32,
                                    tag="st")
                    nc.scalar.copy(st, ps)

                    # ---- s s^T via K=2 matmul at base 0 ----
                    z2 = small.tile([2, D + 1], mybir.dt.float32, tag="z2")
                    nc.vector.memset(z2, 0.0)
                    nc.scalar.copy(z2[0:1, :], st[D:D + 1, :])
                    pso = ps_small.tile([D, D], mybir.dt.float32, tag="ps64")
                    nc.tensor.matmul(pso, z2[:, 0:D], z2[:, 0:D],
                                     start=True, stop=True)

                    # ---- cov A = S/(B-1) - s s^T/(B(B-1)) + eps I ----
                    t1 = small.tile([D, D], mybir.dt.float32, tag="t1")
                    nc.vector.tensor_scalar_mul(t1, st[0:D, 0:D], a_cov)
                    t2 = small.tile([D, D], mybir.dt.float32, tag="t2")
                    nc.vector.tensor_scalar_mul(t2, pso, b_cov)
                    t3 = small.tile([D, D], mybir.dt.float32, tag="t3")
                    nc.vector.tensor_sub(t3, t1, t2)
                    amat = small.tile([D, D], mybir.dt.float32, tag="amat")
                    nc.vector.tensor_add(amat, t3, epsi)

                    # ---- W0 = diag(1/sqrt(diag A)) ----
                    dm = small.tile([D, D], mybir.dt.float32, tag="dm")
                    nc.vector.tensor_mul(dm, amat, ident)
                    dcol = small.tile([D, 1], mybir.dt.float32, tag="dcol")
                    nc.vector.reduce_sum(dcol, dm, axis=mybir.AxisListType.X)
                    rcol = small.tile([D, 1], mybir.dt.float32, tag="rcol")
                    nc.vector.reciprocal(rcol, dcol)
                    wcol = small.tile([D, 1], mybir.dt.float32, tag="wcol")
                    nc.scalar.activation(out=wcol, in_=rcol, func=AF.Sqrt)
                    w = wpool.tile([D, D], mybir.dt.float32, tag="w")
                    nc.vector.tensor_scalar_mul(w, ident, wcol)

                    # ---- Newton iterations ----
                    NIT = 4
                    for it in range(NIT):
                        if it == 0:
                            wt = w          # W0 symmetric (diagonal)
                        else:
                            pst = ps_small.tile([D, D], mybir.dt.float32,
                                                tag="ps64")
                            nc.tensor.transpose(pst, w, ident)
                            wt = wpool.tile([D, D], mybir.dt.float32,
                                            tag="wt")
                            nc.scalar.copy(wt, pst)
                        psh = ps_small.tile([D, D], mybir.dt.float32,
                                            tag="ps64")
                        nc.tensor.matmul(psh, amat, wt, start=True, stop=True)
                        h = small.tile([D, D], mybir.dt.float32, tag="h")
                        nc.scalar.copy(h, psh)
                        psp = ps_small.tile([D, D], mybir.dt.float32,
                                            tag="ps64")
                        nc.tensor.matmul(psp, wt, h, start=True, stop=True)
                        u1 = small.tile([D, D], mybir.dt.float32, tag="u1")
                        nc.vector.tensor_mul(u1, cmask, psp)
                        ct = small.tile([D, D], mybir.dt.float32, tag="ct")
                        nc.vector.tensor_sub(ct, chalf, u1)
                        psd = ps_small.tile([D, D], mybir.dt.float32,
                                            tag="ps64")
                        nc.tensor.matmul(psd, ct, w, start=True, stop=True)
                        wn = wpool.tile([D, D], mybir.dt.float32, tag="w")
                        nc.vector.tensor_add(wn, w, psd)
                        w = wn

                    # ---- final W^T as f32r solve weights ----
                    pst = ps_small.tile([D, D], mybir.dt.float32, tag="ps64")
                    nc.tensor.transpose(pst, w, ident)
                    wtr = wpool.tile([D, D], mybir.dt.float32r, tag="wtr")
                    nc.scalar.copy(wtr, pst)

                    # ---- v = W mu; bias = -v ----
                    mur = small.tile([D, 2], mybir.dt.float32r, tag="mur")
                    nc.scalar.activation(out=mur[:, 1:2],
                                         in_=st[0:D, D:D + 1],
                                         func=AF.Identity, scale=0.0)
                    nc.scalar.activation(out=mur[:, 0:1],
                                         in_=st[0:D, D:D + 1],
                                         func=AF.Identity, scale=inv_b)
                    psv = ps_small.tile([D, 2], mybir.dt.float32, tag="ps64")
                    nc.tensor.matmul(psv, wtr, mur, start=True, stop=True)
                    biask = small.tile([D, 1], mybir.dt.float32, tag="biask")
                    nc.scalar.activation(out=biask, in_=psv[:, 0:1],
                                         func=AF.Identity, scale=-1.0)

                    # ---- solve: Z = W x + bias ----
                    for j in range(NB):
                        psz = ps_z.tile([D, 512], mybir.dt.float32, tag="psz")
                        nc.tensor.matmul(
                            psz, wtr,
                            slab[:, half, 512 * j: 512 * (j + 1)],
                            start=True, stop=True)
                        dst = zpair[half * D:(half + 1) * D,
                                    512 * j:512 * (j + 1)]
                        if half == 0:
                            cp = nc.scalar.activation(out=dst, in_=psz,
                                                      func=AF.Identity,
                                                      bias=biask)
                        else:
                            cp = nc.vector.tensor_scalar_add(dst, psz, biask)
                        outdma_deps.append(cp)

                nc.sync.dma_start(
                    out=d_out.ap()[k0:k0 + 2].rearrange("c d b -> (c d) b"),
                    in_=zpair)

    nc.finalize()
    return nc


def _make_consts():
    ident = np.eye(D, dtype=np.float32)
    cmask = np.triu(np.ones((D, D), np.float32), 1) + 0.5 * ident
    chalf = 0.5 * ident
    epsi = EPS * ident
    return np.concatenate([ident, cmask, chalf, epsi], axis=1)


def _prep_inputs(x):
    """x: [B, D, K] fp32 -> per-core input dicts."""
    consts = _make_consts()
    in_maps = []
    for c in range(NCORES):
        ks = slice(c * KL, (c + 1) * KL)
        xs = np.ascontiguousarray(x[:, :, ks].transpose(2, 1, 0))  # [KL, D, B]
        xt = xs.transpose(0, 2, 1)                                  # [KL, B, D]
        xb = np.empty((KL, B, 66), dtype=ml_dtypes.bfloat16)
        xb[:, :, 0:D] = xt.astype(ml_dtypes.bfloat16)
        xb[:, :, D] = np.float32(1.0)
        xb[:, :, D + 1] = np.float32(0.0)
        in_maps.append({"xs": xs, "xb": xb, "cs": consts})
    return in_maps


def _run(x, trace=False):
    if "nc" not in _cache:
        _cache["nc"] = _build_nc()
    nc = _cache["nc"]
    in_maps = _prep_inputs(np.asarray(x, dtype=np.float32))
    res = run_bass_kernel_spmd(nc, in_maps, core_ids=list(range(NCORES)),
                               trace=trace)
    out = np.empty((B, D, K), dtype=np.float32)
    for c in range(NCORES):
        ks = slice(c * KL, (c + 1) * KL)
        out[:, :, ks] = res.results[c]["out"].transpose(2, 1, 0)
    return out, res


def kernel(x):
    out, _ = _run(x, trace=False)
    return out


# revision 8
# speedup vs baseline: 2.6324x; 2.6324x over previous
"""ClusterNorm1d TRN2 kernel.

Math (per cluster k): mu = mean_b x[b,:,k]; cov = centered second moment;
L = chol(cov + eps I); Z = L^-1 (x - mu).  Output Z transposed back.

Strategy per core (32 clusters): K-sharded across 8 cores, no collectives.
  - stats: bf16 matmuls over a host-prepared [b, (d|1)] tensor, accumulating
    U^T U = [[S, s], [s^T, B]] in fp32 PSUM (32 accumulating matmuls).
  - cov -> W = L^-1 via 4 Newton iterations on the Cholesky manifold:
      P = W A W^T;  C^T = CM o (I - P);  W <- W + C^T^T W
    (CM = triu(1) + 0.5 I).  Converges quadratically; exact-fp32 validated.
  - solve: Z = W x - (W mu) 1^T as float32r matmuls (1 cyc/row @ N=512),
    mean applied as per-partition bias during the PSUM->SBUF copy.
Host supplies x pre-transposed per core as [32, 64, 4096] (f32r) and the
bf16 stats operand [32, 4096, 66] (col 64 = ones, col 65 pad).
"""
import sys
sys.path.insert(0, "/opt/trn_rl_repo")

import numpy as np
import ml_dtypes

import concourse.bass as bass
from concourse import bacc
import concourse.mybir as mybir
import concourse.tile as tile
from concourse.bass_utils import run_bass_kernel_spmd

B, D, K, NCORES = 4096, 64, 256, 8
KL = K // NCORES          # clusters per core
EPS = 1e-4
NB = B // 512             # solve chunks per cluster
AF = mybir.ActivationFunctionType

_cache = {}


def _build_nc(repeat=1):
    nc = bacc.Bacc("TRN2", target_bir_lowering=False, debug=False,
                   num_devices=NCORES)
    d_xs = nc.dram_tensor("xs", [KL, D, B], mybir.dt.float32r,
                          kind="ExternalInput")
    d_xb = nc.dram_tensor("xb", [KL, B, 66], mybir.dt.bfloat16,
                          kind="ExternalInput")
    d_cs = nc.dram_tensor("cs", [D, 4 * D], mybir.dt.float32,
                          kind="ExternalInput")
    d_out = nc.dram_tensor("out", [KL, D, B], mybir.dt.float32,
                           kind="ExternalOutput")

    inv_b = 1.0 / B
    a_cov = 1.0 / (B - 1)
    b_cov = 1.0 / (B * (B - 1.0))

    with tile.TileContext(nc) as tc:
        with tc.tile_pool(name="consts", bufs=1) as consts, \
             tc.tile_pool(name="slab", bufs=2) as slabp, \
             tc.tile_pool(name="upool", bufs=2) as upool, \
             tc.tile_pool(name="zpool", bufs=2) as zpool, \
             tc.tile_pool(name="small", bufs=4) as small, \
             tc.tile_pool(name="wpool", bufs=8) as wpool, \
             tc.tile_pool(name="ps_stat", bufs=2, space="PSUM") as ps_stat, \
             tc.tile_pool(name="ps_small", bufs=4, space="PSUM") as ps_small, \
             tc.tile_pool(name="ps_z", bufs=2, space="PSUM") as ps_z:

            tcs = consts.tile([D, 4 * D], mybir.dt.float32)
            nc.sync.dma_start(out=tcs, in_=d_cs.ap())
            ident = tcs[:, 0:D]
            cmask = tcs[:, D:2 * D]        # triu(1,k=1) + 0.5 I
            chalf = tcs[:, 2 * D:3 * D]    # 0.5 I
            epsi = tcs[:, 3 * D:4 * D]     # EPS * I

            for p0 in range(repeat * (KL // 2)):
                p = p0 % (KL // 2)
                k0, k1 = 2 * p, 2 * p + 1
                # ---- x slab for the pair: [64, 2*4096] f32r ----
                slab = slabp.tile([D, 2, B], mybir.dt.float32r)
                nc.sync.dma_start(
                    out=slab,
                    in_=d_xs.ap()[k0:k0 + 2].rearrange("c d b -> d c b"))

                zpair = zpool.tile([2 * D, B], mybir.dt.float32)
                outdma_deps = []

                for half, kk in enumerate((k0, k1)):
                    # ---- stats ----
                    ub = upool.tile([128, (B // 128) * 66], mybir.dt.bfloat16)
                    nc.sync.dma_start(
                        out=ub,
                        in_=d_xb.ap()[kk].rearrange("(p j) c -> p (j c)",
                                                    p=128))
                    ps = ps_stat.tile([D + 1, D + 1], mybir.dt.float32)
                    for j in range(B // 128):
                        sl = ub[:, 66 * j:66 * j + 65]
                        nc.tensor.matmul(ps, sl, sl, start=(j == 0),
                                         stop=(j == B // 128 - 1))
                    st = small.tile([D + 1, D + 1], mybir.dt.float32,
                                    tag="st")
                    nc.scalar.copy(st, ps)

                    # ---- s s^T via K=2 matmul at base 0 ----
                    z2 = small.tile([2, D + 1], mybir.dt.float32, tag="z2")
                    nc.vector.memset(z2, 0.0)
                    nc.scalar.copy(z2[0:1, :], st[D:D + 1, :])
                    pso = ps_small.tile([D, D], mybir.dt.float32, tag="ps64")
                    nc.tensor.matmul(pso, z2[:, 0:D], z2[:, 0:D],
                                     start=True, stop=True)

                    # ---- cov A = S/(B-1) - s s^T/(B(B-1)) + eps I ----
                    t1 = small.tile([D, D], mybir.dt.float32, tag="t1")
                    nc.vector.tensor_scalar_mul(t1, st[0:D, 0:D], a_cov)
                    t2 = small.tile([D, D], mybir.dt.float32, tag="t2")
                    nc.vector.tensor_scalar_mul(t2, pso, b_cov)
                    t3 = small.tile([D, D], mybir.dt.float32, tag="t3")
                    nc.vector.tensor_sub(t3, t1, t2)
                    amat = small.tile([D, D], mybir.dt.float32, tag="amat")
                    nc.vector.tensor_add(amat, t3, epsi)

                    # ---- W0 = diag(1/sqrt(diag A)) ----
                    dm = small.tile([D, D], mybir.dt.float32, tag="dm")
                    nc.vector.tensor_mul(dm, amat, ident)
                    dcol = small.tile([D, 1], mybir.dt.float32, tag="dcol")
                    nc.vector.reduce_sum(dcol, dm, axis=mybir.AxisListType.X)
                    rcol = small.tile([D, 1], mybir.dt.float32, tag="rcol")
                    nc.vector.reciprocal(rcol, dcol)
                    wcol = small.tile([D, 1], mybir.dt.float32, tag="wcol")
                    nc.scalar.activation(out=wcol, in_=rcol, func=AF.Sqrt)
                    w = wpool.tile([D, D], mybir.dt.float32, tag="w")
                    nc.vector.tensor_scalar_mul(w, ident, wcol)

                    # ---- Newton iterations ----
                    NIT = 4
                    for it in range(NIT):
                        if it == 0:
                            wt = w          # W0 symmetric (diagonal)
                        else:
                            pst = ps_small.tile([D, D], mybir.dt.float32,
                                                tag="ps64")
                            nc.tensor.transpose(pst, w, ident)
                            wt = wpool.tile([D, D], mybir.dt.float32,
                                            tag="wt")
                            nc.scalar.copy(wt, pst)
                        psh = ps_small.tile([D, D], mybir.dt.float32,
                                            tag="ps64")
                        nc.tensor.matmul(psh, amat, wt, start=True, stop=True)
                        h = small.tile([D, D], mybir.dt.float32, tag="h")
                        nc.scalar.copy(h, psh)
                        psp = ps_small.tile([D, D], mybir.dt.float32,
                                            tag="ps64")
                        nc.tensor.matmul(psp, wt, h, start=True, stop=True)
                        u1 = small.tile([D, D], mybir.dt.float32, tag="u1")
                        nc.vector.tensor_mul(u1, cmask, psp)
                        ct = small.tile([D, D], mybir.dt.float32, tag="ct")
                        nc.vector.tensor_sub(ct, chalf, u1)
                        psd = ps_small.tile([D, D], mybir.dt.float32,
                                            tag="ps64")
                        nc.tensor.matmul(psd, ct, w, start=True, stop=True)
                        wn = wpool.tile([D, D], mybir.dt.float32, tag="w")
                        nc.vector.tensor_add(wn, w, psd)
                        w = wn

                    # ---- final W^T as f32r solve weights ----
                    pst = ps_small.tile([D, D], mybir.dt.float32, tag="ps64")
                    nc.tensor.transpose(pst, w, ident)
                    wtr = wpool.tile([D, D], mybir.dt.float32r, tag="wtr")
                    nc.scalar.copy(wtr, pst)

                    # ---- v = W mu; bias = -v ----
                    mur = small.tile([D, 2], mybir.dt.float32r, tag="mur")
                    nc.scalar.activation(out=mur[:, 1:2],
                                         in_=st[0:D, D:D + 1],
                                         func=AF.Identity, scale=0.0)
                    nc.scalar.activation(out=mur[:, 0:1],
                                         in_=st[0:D, D:D + 1],
                                         func=AF.Identity, scale=inv_b)
                    psv = ps_small.tile([D, 2], mybir.dt.float32, tag="ps64")
                    nc.tensor.matmul(psv, wtr, mur, start=True, stop=True)
                    biask = small.tile([D, 1], mybir.dt.float32, tag="biask")
                    nc.scalar.activation(out=biask, in_=psv[:, 0:1],
                                         func=AF.Identity, scale=-1.0)

                    # ---- solve: Z = W x + bias ----
                    for j in range(NB):
                        psz = ps_z.tile([D, 512], mybir.dt.float32, tag="psz")
                        nc.tensor.matmul(
                            psz, wtr,
                            slab[:, half, 512 * j: 512 * (j + 1)],
                            start=True, stop=True)
                        dst = zpair[half * D:(half + 1) * D,
                                    512 * j:512 * (j + 1)]
                        if half == 0:
                            cp = nc.scalar.activation(out=dst, in_=psz,
                                                      func=AF.Identity,
                                                      bias=biask)
                        else:
                            cp = nc.vector.tensor_scalar_add(dst, psz, biask)
                        outdma_deps.append(cp)

                nc.sync.dma_start(
                    out=d_out.ap()[k0:k0 + 2].rearrange("c d b -> (c d) b"),
                    in_=zpair)

    nc.finalize()
    return nc


def _make_consts():
    ident = np.eye(D, dtype=np.float32)
    cmask = np.triu(np.ones((D, D), np.float32), 1) + 0.5 * ident
    chalf = 0.5 * ident
    epsi = EPS * ident
    return np.concatenate([ident, cmask, chalf, epsi], axis=1)


def _prep_inputs(x):
    """x: [B, D, K] fp32 -> per-core input dicts."""
    consts = _make_consts()
    in_maps = []
    for c in range(NCORES):
        ks = slice(c * KL, (c + 1) * KL)
        xs = np.ascontiguousarray(x[:, :, ks].transpose(2, 1, 0))  # [KL, D, B]
        xt = xs.transpose(0, 2, 1)                                  # [KL, B, D]
        xb = np.empty((KL, B, 66), dtype=ml_dtypes.bfloat16)
        xb[:, :, 0:D] = xt.astype(ml_dtypes.bfloat16)
        xb[:, :, D] = np.float32(1.0)
        xb[:, :, D + 1] = np.float32(0.0)
        in_maps.append({"xs": xs, "xb": xb, "cs": consts})
    return in_maps


def _run(x, trace=False):
    if "nc" not in _cache:
        _cache["nc"] = _build_nc()
    nc = _cache["nc"]
    in_maps = _prep_inputs(np.asarray(x, dtype=np.float32))
    res = run_bass_kernel_spmd(nc, in_maps, core_ids=list(range(NCORES)),
                               trace=trace)
    out = np.empty((B, D, K), dtype=np.float32)
    for c in range(NCORES):
        ks = slice(c * KL, (c + 1) * KL)
        out[:, :, ks] = res.results[c]["out"].transpose(2, 1, 0)
    return out, res


def kernel(x):
    out, _ = _run(x, trace=False)
    return out
